# revision 1
# baseline (speedup 1.0000x reference)
"""Causal self-attention (B=2, T=4096, C=768, H=12) on 8 trn2 NeuronCores.

Sharding: data-parallel on batch (cores 0-3 -> batch 0, cores 4-7 -> batch 1),
tensor-parallel on heads (3 heads per core).  Each core computes qkv for its
3 heads, causal flash-style attention, and a partial output projection
(its heads' rows of w_proj); the host sums the 4 partials per batch.

All matmuls run in fp32r (TF32-like, 13-bit mantissa, full PE speed).
Attention is computed in a transposed layout (S^T tiles = K_tile^T x Q) so
softmax sums come from a ones-column appended to V, and no transposes are
needed in the inner loop.

Perf structure (v7): the kernel is ScalarE(exp)-latency-bound, so the
attention inner loop batches two 128x512 S-tiles into one [128,1024] exp
(halving ACT instruction count), interleaves heads 0/1's k-loops at pair
granularity to keep independent exps in flight, double-buffers both the
projection output staging and the qkv input staging, and keeps ScalarE
free of copies so attention exps are never queued behind them (measured
regression when x-rounding ran on ScalarE).  The final diagonal pair of
each query block runs at half width (its keys are invisible to queries
in columns < 256), and the Y^T staging buffer is double-buffered so each
query block's normalize tail overlaps the next block's accumulation.
Measured ~0.65-0.68 ms per iteration on HW (repeat-16 vs repeat-8 wall
differencing), down from ~0.88-0.94 ms for the unpipelined version.
"""

import sys

if '/opt/trn_rl_repo' not in sys.path:
    sys.path.insert(0, '/opt/trn_rl_repo')

import numpy as np

import concourse.bacc as bacc
import concourse.mybir as mybir
import concourse.tile as tile
from concourse.masks import make_identity

dt = mybir.dt
F32 = dt.float32
F32R = dt.float32r

N_EMBD = 768
N_HEADS = 12
HEAD_DIM = 64
B = 2
T_FULL = 4096
N_CORES = 8
HEADS_PER_CORE = N_HEADS // (N_CORES // B)  # 3

TOK_CHUNK = 256   # qkv phase token chunk
QSB = 512         # attention query superblock
KT = 128          # key tile (contraction for P@V)
CCHUNKS = N_EMBD // 128  # 6 contraction chunks


BUFS = {"pbig": 2, "py": 2, "psmall": 2, "ptp": 2, "ysb": 2, "yqn": 1, "xr": 2, "xs": 2}
C_MODE = "full"  # debug knob: full | noy | notail | nomask


def build_nc(T=T_FULL, repeat=1, phases=('B','B2','C','D')):
    """Build the per-core Bass program.  Same program runs SPMD on all 8
    cores; per-core data (x^T of its batch, its heads' weight slices) comes
    via the input map."""
    nc = bacc.Bacc(None, target_bir_lowering=False, debug=False)

    n_kt = T // KT
    n_qsb = T // QSB
    n_tok = T // 128

    XT = nc.dram_tensor("xt", [N_EMBD, T], F32, kind="ExternalInput")
    WQ01 = nc.dram_tensor("wq01", [N_EMBD, 128], F32, kind="ExternalInput")
    WK01 = nc.dram_tensor("wk01", [N_EMBD, 128], F32, kind="ExternalInput")
    WV01 = nc.dram_tensor("wv01", [N_EMBD, 128], F32, kind="ExternalInput")
    WQV2 = nc.dram_tensor("wqv2", [N_EMBD, 128], F32, kind="ExternalInput")
    WK2 = nc.dram_tensor("wk2", [N_EMBD, 64], F32, kind="ExternalInput")
    WP1 = nc.dram_tensor("wp1", [128, N_EMBD], F32, kind="ExternalInput")
    WP2 = nc.dram_tensor("wp2", [64, N_EMBD], F32, kind="ExternalInput")
    Y = nc.dram_tensor("y", [T, N_EMBD], F32, kind="ExternalOutput")

    xt_ap = XT.ap().rearrange("(c p) t -> p c t", p=128)

    with tile.TileContext(nc) as tc:
        with (
            tc.tile_pool(name="const", bufs=1) as const_pool,
            tc.tile_pool(name="wpool", bufs=1) as wpool,
            tc.tile_pool(name="wstage", bufs=1) as wstage,
            tc.tile_pool(name="qkvt", bufs=1) as qkvt,
            tc.tile_pool(name="vsb", bufs=1) as vsb_pool,
            tc.tile_pool(name="ynt", bufs=1) as ynt_pool,
            tc.tile_pool(name="xs", bufs=BUFS["xs"]) as xs_pool,
            tc.tile_pool(name="xr", bufs=BUFS["xr"]) as xr_pool,
            tc.tile_pool(name="ptp", bufs=BUFS["ptp"]) as pt_pool,
            tc.tile_pool(name="ysb", bufs=BUFS["ysb"]) as ysb_pool,
            tc.tile_pool(name="rp", bufs=4) as r_pool,
            tc.tile_pool(name="yout", bufs=2) as yout_pool,
            tc.tile_pool(name="yqn", bufs=BUFS["yqn"]) as yqn_pool,
            tc.tile_pool(name="pbig", bufs=BUFS["pbig"], space="PSUM") as pbig,
            tc.tile_pool(name="py", bufs=BUFS["py"], space="PSUM") as py_pool,
            tc.tile_pool(name="psmall", bufs=BUFS["psmall"], space="PSUM") as psmall,
        ):
            # ---- constants (built in fp32 scratch, rounded to fp32r) ----
            ident_f = const_pool.tile([128, 128], F32)
            make_identity(nc, ident_f)
            ident = const_pool.tile([128, 128], F32R)
            nc.vector.tensor_copy(out=ident, in_=ident_f)
            # causal mask master: M[i, c] = 1.0 iff c >= i + 384.
            # slice [384-d : 896-d] gives tile-mask for diag offset d.
            mask_f = wstage.tile([128, QSB + 384], F32, tag="wst")
            nc.gpsimd.memset(mask_f, 1.0)
            nc.gpsimd.affine_select(
                out=mask_f, in_=mask_f,
                compare_op=mybir.AluOpType.is_ge,
                fill=0.0, base=-384, channel_multiplier=-1,
                pattern=[[1, QSB + 384]],
            )
            mask = const_pool.tile([128, QSB + 384], F32R)
            nc.vector.tensor_copy(out=mask, in_=mask_f)

            # ---- weights: load + round to fp32r ----
            def load_w(src_ap, shape_r, tag):
                st = wstage.tile(shape_r, F32, tag="wst")
                nc.sync.dma_start(out=st, in_=src_ap)
                rt = wpool.tile(shape_r, F32R, tag=tag)
                nc.vector.tensor_copy(out=rt, in_=st)
                return rt

            wq01r = load_w(WQ01.ap().rearrange("(c p) m -> p c m", p=128), [128, CCHUNKS, 128], tag="wq01r")
            wk01r = load_w(WK01.ap().rearrange("(c p) m -> p c m", p=128), [128, CCHUNKS, 128], tag="wk01r")
            wv01r = load_w(WV01.ap().rearrange("(c p) m -> p c m", p=128), [128, CCHUNKS, 128], tag="wv01r")
            wqv2r = load_w(WQV2.ap().rearrange("(c p) m -> p c m", p=128), [128, CCHUNKS, 128], tag="wqv2r")
            wk2r = load_w(WK2.ap().rearrange("(c p) m -> p c m", p=128), [128, CCHUNKS, 64], tag="wk2r")
            wp1r = load_w(WP1.ap(), [128, N_EMBD], tag="wp1r")
            wp2r = load_w(WP2.ap(), [64, N_EMBD], tag="wp2r")

            # ---- persistent activations ----
            QT01 = qkvt.tile([128, T], F32R, tag="qt01")
            KT01 = qkvt.tile([128, T], F32R, tag="kt01")
            VT01 = qkvt.tile([128, T], F32R, tag="vt01")
            QV2 = qkvt.tile([128, T], F32R, tag="qv2")   # q_h2 rows 0:64, v_h2 rows 64:128
            KT2 = qkvt.tile([64, T], F32R, tag="kt2")
            Vsb = vsb_pool.tile([128, n_kt, HEADS_PER_CORE, 65], F32R)
            YnT01 = ynt_pool.tile([128, T], F32R, tag="ynt01")
            YnT2 = ynt_pool.tile([64, T], F32R, tag="ynt2")

            ones_f = const_pool.tile([128, n_kt * HEADS_PER_CORE], F32)
            nc.vector.memset(ones_f, 1.0)
            nc.vector.tensor_copy(
                out=Vsb[:, :, :, 64:65].rearrange("p a b c -> p (a b c)"),
                in_=ones_f)

            for _ in range(repeat):
                # ================= phase B: qkv projections ================
                # out tensors are [M, tok] with M = packed head-dim rows:
                #   QT01 = [q_h0; q_h1], KT01 = [k_h0; k_h1], VT01 = [v_h0; v_h1],
                #   QK2 = [q_h2; k_h2], VT2 = [v_h2]
                qkv_jobs = [
                    (wq01r, QT01, 128), (wk01r, KT01, 128), (wv01r, VT01, 128),
                    (wqv2r, QV2, 128), (wk2r, KT2, 64),
                ]
                for ch in range(T // TOK_CHUNK if 'B' in phases else 0):
                    sl = slice(ch * TOK_CHUNK, (ch + 1) * TOK_CHUNK)
                    xs = xs_pool.tile([128, CCHUNKS, TOK_CHUNK], F32)
                    nc.sync.dma_start(out=xs, in_=xt_ap[:, :, sl])
                    xr = xr_pool.tile([128, CCHUNKS, TOK_CHUNK], F32R)
                    nc.vector.tensor_copy(out=xr, in_=xs)
                    for wt, out_sb, m in qkv_jobs:
                        ps = pbig.tile([128, TOK_CHUNK], F32, tag="big")
                        for c in range(CCHUNKS):
                            nc.tensor.matmul(
                                ps[0:m, :], wt[:, c, 0:m], xr[:, c, :],
                                start=(c == 0), stop=(c == CCHUNKS - 1),
                            )
                        nc.vector.tensor_copy(out=out_sb[0:m, sl], in_=ps[0:m, :])

                # ========== phase B2: V^T -> V (keys-major) transposes =====
                for h in range(HEADS_PER_CORE if 'B2' in phases else 0):
                    for kt in range(n_kt):
                        ks = slice(kt * KT, (kt + 1) * KT)
                        if h == 0:
                            src, idn = VT01[0:64, ks], ident[0:64, 0:64]
                        elif h == 1:
                            src, idn = VT01[64:128, ks], ident[64:128, 64:128]
                        else:
                            src, idn = QV2[64:128, ks], ident[64:128, 64:128]
                        pv = psmall.tile([128, 64], F32R, tag="small")
                        nc.tensor.transpose(pv[:, 0:64], src, idn)
                        nc.vector.tensor_copy(out=Vsb[:, kt, h, 0:64], in_=pv[:, 0:64])

                # ================= phase C: attention ======================
                head_qk = [
                    (QT01[0:64, :], KT01[0:64, :]),
                    (QT01[64:128, :], KT01[64:128, :]),
                    (QV2[0:64, :], KT2[0:64, :]),
                ]
                def attend_kloop_gen(h, qs, qt_h, kt_h, nkt_q, yps):
                    qsl = slice(qs * QSB, (qs + 1) * QSB)
                    for kt2 in range(0, nkt_q, 2):
                        yield
                        # the final (diagonal) pair has delta = (256, 384):
                        # queries in columns [0, 256) see none of its keys, so
                        # compute it at half width (columns 256:512 only).
                        last = (kt2 == nkt_q - 2)
                        q0 = QSB // 2 if last else 0
                        wsl = slice(q0, QSB)
                        sps2 = pbig.tile([128, 2, QSB], F32, tag="big")
                        for j in range(2):
                            kt = kt2 + j
                            ksl = slice(kt * KT, (kt + 1) * KT)
                            nc.tensor.matmul(sps2[:, j, wsl], kt_h[:, ksl],
                                             qt_h[:, qs * QSB + q0:(qs + 1) * QSB],
                                             start=True, stop=True)
                        pt2 = pt_pool.tile([128, 2, QSB], F32R)
                        nc.scalar.activation(
                            out=pt2[:, :, wsl], in_=sps2[:, :, wsl],
                            func=mybir.ActivationFunctionType.Exp,
                            scale=float(HEAD_DIM) ** -0.5,
                        )
                        for j in range(2):
                            kt = kt2 + j
                            delta = kt * KT - qs * QSB
                            if delta >= -KT + 1 and C_MODE != "nomask":
                                nc.vector.tensor_mul(
                                    pt2[:, j, wsl], pt2[:, j, wsl],
                                    mask[:, 384 - delta + q0: 384 - delta + QSB])
                        if C_MODE == "noy":
                            continue
                        for j in range(2):
                            kt = kt2 + j
                            nc.tensor.matmul(yps[:, wsl], Vsb[:, kt, h, :],
                                             pt2[:, j, wsl],
                                             start=(kt == 0),
                                             stop=(kt == nkt_q - 1))

                def attend_pair(qs, hs):
                    """Interleave the k-loops of the heads in `hs` at pair
                    granularity so ScalarE always has an independent exp
                    ready (hides cross-engine latency)."""
                    nkt_q = (qs + 1) * (QSB // KT)
                    ypss = {}
                    for h in hs:
                        ypss[h] = py_pool.tile([65, QSB], F32, tag="y",
                                               name=f"yps{h}")
                    gens = {h: attend_kloop_gen(h, qs, *head_qk[h], nkt_q, ypss[h])
                            for h in hs}
                    live = dict(gens)
                    while live:
                        for h in list(live):
                            try:
                                next(live[h])
                            except StopIteration:
                                del live[h]
                    return ypss

                def finish_qsb(h, qs, yps):
                        if C_MODE in ("noy", "notail"):
                            return
                        ysb = ysb_pool.tile([65, QSB], F32)
                        nc.vector.tensor_copy(out=ysb, in_=yps)
                        # transpose + normalize 128-query chunks
                        for qt in range(QSB // 128):
                            csl = slice(qs * QSB + qt * 128, qs * QSB + (qt + 1) * 128)
                            pt1 = psmall.tile([128, 65], F32, tag="small")
                            nc.tensor.transpose(
                                pt1, ysb[:, qt * 128:(qt + 1) * 128], ident_f[0:65, 0:65])
                            rr = r_pool.tile([128, 1], F32)
                            nc.vector.reciprocal(rr, pt1[:, 64:65])
                            yqn = yqn_pool.tile([128, 64], F32R)
                            nc.vector.tensor_scalar_mul(yqn, pt1[:, 0:64], rr)
                            pt2 = psmall.tile([64, 128], F32R, tag="small")
                            nc.tensor.transpose(pt2, yqn, ident)
                            if h == 0:
                                dst = YnT01[0:64, csl]
                            elif h == 1:
                                dst = YnT01[64:128, csl]
                            else:
                                dst = YnT2[0:64, csl]
                            nc.vector.tensor_copy(out=dst, in_=pt2)

                def attend_qs_pair(h, qs_list):
                    """Interleave one head's k-loops across two query blocks
                    (used for head 2, which has no partner head)."""
                    gens, ypss = {}, {}
                    for qs in qs_list:
                        nkt_q = (qs + 1) * (QSB // KT)
                        ypss[qs] = py_pool.tile([65, QSB], F32, tag="y",
                                                name=f"yps2_{qs}")
                        gens[qs] = attend_kloop_gen(h, qs, *head_qk[h],
                                                    nkt_q, ypss[qs])
                    live = dict(gens)
                    while live:
                        for qs in list(live):
                            try:
                                next(live[qs])
                            except StopIteration:
                                del live[qs]
                    return ypss

                if 'C' in phases:
                    for qs in range(n_qsb):
                        for hs in ((0, 1), (2,)):
                            ypss = attend_pair(qs, hs)
                            for h in hs:
                                finish_qsb(h, qs, ypss[h])

                # ================= phase D: partial projection =============
                for tt in range(n_tok if 'D' in phases else 0):
                    tsl = slice(tt * 128, (tt + 1) * 128)
                    yo = yout_pool.tile([128, N_EMBD], F32)
                    for c0, ncols in ((0, 512), (512, 256)):
                        pp = pbig.tile([128, 512], F32, tag="big")
                        nc.tensor.matmul(pp[:, 0:ncols], YnT01[:, tsl],
                                         wp1r[:, c0:c0 + ncols], start=True, stop=False)
                        nc.tensor.matmul(pp[:, 0:ncols], YnT2[0:64, tsl],
                                         wp2r[0:64, c0:c0 + ncols], start=False, stop=True)
                        nc.vector.tensor_copy(out=yo[:, c0:c0 + ncols], in_=pp[:, 0:ncols])
                    nc.sync.dma_start(out=Y.ap()[tsl, :], in_=yo)

    nc.compile()
    return nc


def make_in_maps(x, w_qkv, w_proj, T=T_FULL):
    """Per-core input dicts from full inputs (numpy)."""
    x = np.asarray(x, dtype=np.float32)
    w_qkv = np.asarray(w_qkv, dtype=np.float32)
    w_proj = np.asarray(w_proj, dtype=np.float32)
    cores_per_batch = N_CORES // B
    xt_b = [np.ascontiguousarray(x[b].T) for b in range(B)]  # [768, T]
    in_maps = []
    for core in range(N_CORES):
        b = core // cores_per_batch
        h0 = (core % cores_per_batch) * HEADS_PER_CORE
        h1, h2 = h0 + 1, h0 + 2
        col = lambda kind, h: w_qkv[:, kind * N_EMBD + h * HEAD_DIM:
                                    kind * N_EMBD + (h + 1) * HEAD_DIM]
        in_maps.append({
            "xt": xt_b[b],
            "wq01": np.ascontiguousarray(np.concatenate([col(0, h0), col(0, h1)], axis=1)),
            "wk01": np.ascontiguousarray(np.concatenate([col(1, h0), col(1, h1)], axis=1)),
            "wv01": np.ascontiguousarray(np.concatenate([col(2, h0), col(2, h1)], axis=1)),
            "wqv2": np.ascontiguousarray(np.concatenate([col(0, h2), col(2, h2)], axis=1)),
            "wk2": np.ascontiguousarray(col(1, h2)),
            "wp1": np.ascontiguousarray(w_proj[h0 * HEAD_DIM:(h1 + 1) * HEAD_DIM, :]),
            "wp2": np.ascontiguousarray(w_proj[h2 * HEAD_DIM:(h2 + 1) * HEAD_DIM, :]),
        })
    return in_maps


def gather_output(results, T=T_FULL):
    cores_per_batch = N_CORES // B
    out = np.empty((B, T, N_EMBD), dtype=np.float32)
    for b in range(B):
        parts = [results[b * cores_per_batch + j]["y"] for j in range(cores_per_batch)]
        out[b] = parts[0] + parts[1] + parts[2] + parts[3]
    return out


_CACHE = {}


def _get_nc(T=T_FULL, repeat=1):
    key = (T, repeat)
    if key not in _CACHE:
        _CACHE[key] = build_nc(T, repeat)
    return _CACHE[key]


def kernel(x, w_qkv, w_proj):
    import time as _time
    from concourse.bass_utils import run_bass_kernel_spmd
    T = x.shape[1]
    nc = _get_nc(T)
    in_maps = make_in_maps(x, w_qkv, w_proj, T)
    last_err = None
    for attempt in range(3):
        try:
            res = run_bass_kernel_spmd(nc, in_maps, list(range(N_CORES)))
            return gather_output(res.results, T)
        except Exception as e:  # transient device wedge: retry after a pause
            last_err = e
            _time.sleep(20 * (attempt + 1))
    raise last_err

